# revision 19
# baseline (speedup 1.0000x reference)
import os
import numpy as np

# ---- problem constants (hardcoded; kernel.py must be self-contained) ----
IMG, WS, SHIFT = 32, 8, 4
C, HEADS, DEPTH = 512, 16, 24
E_DIM, N_E, B = 256, 8192, 8
L = IMG * IMG            # 1024
NW = WS * WS             # 64 tokens per window
HD = C // HEADS          # 32
NWIN = (IMG // WS) ** 2  # 16
FH = 4 * C               # 2048
P = 128
VBLK = HD + 2            # 34 (32 vals + softmax-denominator col + pad)
VW = HEADS * VBLK        # 544
NT = L // P              # 8 token tiles
KC = C // P              # 4 k-tiles over C
KE = E_DIM // P          # 2 k-tiles over E_DIM
SCALE = HD ** -0.5

_DEPTH = int(os.environ.get("BT_DEPTH", DEPTH))
_NCORES = int(os.environ.get("BT_NCORES", 8))


# ---- host-side helpers (mirror reference.py) ----
def _rel_index():
    coords = np.stack(np.meshgrid(np.arange(WS), np.arange(WS), indexing='ij'))
    cf = coords.reshape(2, -1)
    rel = (cf[:, :, None] - cf[:, None, :]).transpose(1, 2, 0)
    rel[:, :, 0] += WS - 1
    rel[:, :, 1] += WS - 1
    rel[:, :, 0] *= 2 * WS - 1
    return rel.sum(-1)  # [NW, NW] int


def _shift_mask():
    img = np.zeros((IMG, IMG), np.float32)
    cnt = 0
    sl = (slice(0, -WS), slice(-WS, -SHIFT), slice(-SHIFT, None))
    for hs in sl:
        for ws_ in sl:
            img[hs, ws_] = cnt
            cnt += 1
    win = img.reshape(IMG // WS, WS, IMG // WS, WS).transpose(0, 2, 1, 3).reshape(-1, NW)
    diff = win[:, None, :] - win[:, :, None]
    return np.where(diff != 0, -100.0, 0.0).astype(np.float32)  # [NWIN, NW, NW]


def _win_perm():
    """raster token index -> window-major position; perm[t_raster] = t_dev"""
    t = np.arange(L).reshape(IMG, IMG)
    wm = t.reshape(IMG // WS, WS, IMG // WS, WS).transpose(0, 2, 1, 3).reshape(-1)
    # wm[t_dev] = t_raster
    inv = np.empty(L, np.int64)
    inv[wm] = np.arange(L)
    return wm, inv  # wm: dev->raster, inv: raster->dev


_WM, _WM_INV = _win_perm()
_REL = _rel_index()
_MASK = _shift_mask()


def _prepare(inputs):
    import ml_dtypes
    BF = ml_dtypes.bfloat16
    f32 = lambda a: np.ascontiguousarray(a, dtype=np.float32)
    bf = lambda a: np.ascontiguousarray(np.asarray(a, np.float32).astype(BF))
    x = np.asarray(inputs['x'], np.float32)          # [B, L, E]
    dec_w = np.asarray(inputs['dec_w'], np.float32)  # [C, E]
    dec_b = np.asarray(inputs['dec_b'], np.float32)
    pos = np.asarray(inputs['pos_embed'], np.float32)[0]  # [L, C]
    n1w = np.asarray(inputs['n1w'], np.float32)
    n1b = np.asarray(inputs['n1b'], np.float32)
    qkv_w = np.asarray(inputs['qkv_w'], np.float32)
    qkv_b = np.asarray(inputs['qkv_b'], np.float32)
    proj_w = np.asarray(inputs['proj_w'], np.float32)
    proj_b = np.asarray(inputs['proj_b'], np.float32)
    rel_bias = np.asarray(inputs['rel_bias'], np.float32)
    n2w = np.asarray(inputs['n2w'], np.float32)
    n2b = np.asarray(inputs['n2b'], np.float32)
    fc1_w = np.asarray(inputs['fc1_w'], np.float32)
    fc1_b = np.asarray(inputs['fc1_b'], np.float32)
    fc2_w = np.asarray(inputs['fc2_w'], np.float32)
    fc2_b = np.asarray(inputs['fc2_b'], np.float32)
    normf_w = np.asarray(inputs['normf_w'], np.float32)
    normf_b = np.asarray(inputs['normf_b'], np.float32)
    pred_w = np.asarray(inputs['pred_w'], np.float32)
    pred_b = np.asarray(inputs['pred_b'], np.float32)

    D = _DEPTH
    sh = {}
    # dec: out token-major: lhsT = xT (per core), rhs = dec_w.T [E, C]
    sh['decw'] = bf(dec_w.T)                                     # [E, C]
    # pos+dec_b in device (window-major) token order
    sh['posb'] = f32((pos + dec_b[None, :])[_WM])                # [L, C]

    wqk = np.empty((D, C, 2 * C), np.float32)
    bqk = np.zeros((D, P, 8), np.float32)
    wvp = np.zeros((D, C, VW), np.float32)
    vb = np.zeros((D, P, VW), np.float32)
    ab = np.empty((D, NT, 4, P, 4, NW), np.float32)  # (wp, rg, 2w*tk, hi, tq)
    wp_ = np.empty((D, C, C), np.float32)
    pb = np.empty((D, P, C), np.float32)
    f1 = np.empty((D, C, FH), np.float32)
    f1b = np.empty((D, P, FH // P), np.float32)
    f2 = np.empty((D, FH, C), np.float32)
    f2b = np.empty((D, P, C), np.float32)

    for i in range(D):
        Wm = qkv_w[i] * n1w[i][None, :]           # [3C, C]
        bm = qkv_w[i] @ n1b[i] + qkv_b[i]         # [3C]
        Wm = Wm.copy()
        bm = bm.copy()
        Wm[:C] *= SCALE
        bm[:C] *= SCALE
        wqk[i] = Wm[:2 * C].T                     # [C, 2C]
        bqk[i] = bm[:2 * C].reshape(8, P).T       # bias for out-channel tile mo at [:, mo]
        # v with padded 34-blocks; ones column added via vb during evacuation
        for h in range(HEADS):
            wvp[i][:, h * VBLK:h * VBLK + HD] = Wm[2 * C + h * HD:2 * C + (h + 1) * HD].T
            vb[i][:, h * VBLK:h * VBLK + HD] = bm[2 * C + h * HD:2 * C + (h + 1) * HD][None, :]
            vb[i][:, h * VBLK + HD] = 1.0
        # attention additive bias [tk, tq] per (win, head)
        bias = rel_bias[i][_REL]                  # [tq, tk, HEADS]
        shift = (i % 2) == 1
        for w in range(NWIN):
            for h in range(HEADS):
                a = bias[:, :, h].T               # [tk, tq]
                if shift:
                    a = a + _MASK[w].T
                wp2, w01 = w // 2, w % 2
                rg, hi = h % 4, h // 4
                ab[i, wp2, rg, w01 * NW:(w01 + 1) * NW, hi, :] = a
        wp_[i] = proj_w[i].T
        pb[i] = np.broadcast_to(proj_b[i][None, :], (P, C))
        F1m = (fc1_w[i] * n2w[i][None, :]).T      # [C, FH]
        f1[i] = F1m
        f1b[i] = (fc1_w[i] @ n2b[i] + fc1_b[i]).reshape(FH // P, P).T
        f2[i] = fc2_w[i].T
        f2b[i] = np.broadcast_to(fc2_b[i][None, :], (P, C))

    sh['wqk'] = bf(wqk)
    sh['bqk'] = f32(bqk)
    sh['wvp'] = bf(wvp)
    sh['vb'] = bf(vb)
    sh['ab'] = np.ascontiguousarray(ab.reshape(D, NT, 4, P, 4 * NW).astype(BF))
    sh['wp'] = bf(wp_)
    sh['pb'] = f32(pb)
    sh['f1'] = bf(f1)
    sh['f1b'] = f32(f1b)
    sh['f2'] = bf(f2)
    sh['f2b'] = f32(f2b)
    sh['nfw'] = f32(normf_w.reshape(KC, P).T)     # [P, KC]
    sh['nfb'] = f32(normf_b.reshape(KC, P).T)
    sh['pw'] = bf(pred_w.T)                       # [C, N_E]
    sh['pwb'] = f32(pred_b.reshape(N_E // P, P).T)  # [P, 64]
    # per-core xT in device token order: [E, L]
    xts = [bf(x[c][_WM].T) for c in range(B)]
    return sh, xts


# ---- device program ----
_BUILD_CACHE = {}


def _build(skip_bias):
    key = (_DEPTH, skip_bias)
    if key in _BUILD_CACHE:
        return _BUILD_CACHE[key]
    import concourse.bass as bass
    import concourse.mybir as mybir
    import concourse.tile as tile
    from concourse import bacc
    from concourse.masks import make_identity
    from contextlib import ExitStack

    F32 = mybir.dt.float32
    I32 = mybir.dt.int32
    BF16 = mybir.dt.bfloat16
    AF = mybir.ActivationFunctionType
    ALU = mybir.AluOpType
    AX = mybir.AxisListType
    D = _DEPTH

    nc = bacc.Bacc("TRN2", target_bir_lowering=False, debug=False, num_devices=_NCORES)

    dr = {}
    def din(name, shape, dt):
        dr[name] = nc.dram_tensor(name, list(shape), dt, kind="ExternalInput").ap()
    din('xT', (E_DIM, L), BF16)
    din('decw', (E_DIM, C), BF16)
    din('posb', (L, C), F32)
    din('wqk', (D, C, 2 * C), BF16)
    din('bqk', (D, P, 8), F32)
    din('wvp', (D, C, VW), BF16)
    din('vb', (D, P, VW), BF16)
    din('ab', (D, NT, 4, P, 4 * NW), BF16)
    din('wp', (D, C, C), BF16)
    din('pb', (D, P, C), F32)
    din('f1', (D, C, FH), BF16)
    din('f1b', (D, P, FH // P), F32)
    din('f2', (D, FH, C), BF16)
    din('f2b', (D, P, C), F32)
    din('nfw', (P, KC), F32)
    din('nfb', (P, KC), F32)
    din('pw', (C, N_E), BF16)
    din('pwb', (P, N_E // P), F32)
    outT = nc.dram_tensor("outT", [N_E, L], F32, kind="ExternalOutput").ap()

    with tile.TileContext(nc) as tc, ExitStack() as ES:
        # ---------- persistent SBUF pools ----------
        cst = ES.enter_context(tc.tile_pool(name="cst", bufs=1))
        ident_f = cst.tile([P, P], F32)
        make_identity(nc, ident_f)
        ident_b = cst.tile([P, P], BF16)
        nc.scalar.copy(ident_b[:], ident_f[:])

        xp = ES.enter_context(tc.tile_pool(name="xp", bufs=1))
        hp = ES.enter_context(tc.tile_pool(name="hp", bufs=2))
        hTp = ES.enter_context(tc.tile_pool(name="hTp", bufs=4))
        qkp = ES.enter_context(tc.tile_pool(name="qkp", bufs=1))
        vp = ES.enter_context(tc.tile_pool(name="vp", bufs=1))
        attp = ES.enter_context(tc.tile_pool(name="attp", bufs=1))
        ppool = ES.enter_context(tc.tile_pool(name="ppool", bufs=3))
        abp = ES.enter_context(tc.tile_pool(name="abp", bufs=5))
        stp = ES.enter_context(tc.tile_pool(name="stp", bufs=2))
        recp = ES.enter_context(tc.tile_pool(name="recp", bufs=2))
        wqkp = ES.enter_context(tc.tile_pool(name="wqkp", bufs=4))
        wvpp = ES.enter_context(tc.tile_pool(name="wvpp", bufs=1))
        wpp = ES.enter_context(tc.tile_pool(name="wpp", bufs=1))
        f1p = ES.enter_context(tc.tile_pool(name="f1p", bufs=1))
        f2p = ES.enter_context(tc.tile_pool(name="f2p", bufs=1))
        gp = ES.enter_context(tc.tile_pool(name="gp", bufs=2))
        bp = ES.enter_context(tc.tile_pool(name="bp", bufs=2))
        outp = ES.enter_context(tc.tile_pool(name="outp", bufs=3))
        pwp = ES.enter_context(tc.tile_pool(name="pwp", bufs=3))

        x = xp.tile([P, NT, C], F32)

        # ---------- LN helpers (stats on DVE, rsqrt via bit-trick on DVE) ----------
        def ln_stats(src_slice, mvg, tt):
            bns = stp.tile([P, 6], F32, name="bns", tag=f"bns{tt % 2}")
            nc.vector.bn_stats(bns[:], src_slice)
            nc.vector.bn_aggr(mvg[:, tt], bns[:])

        def ln_rstd_group(mvg, rstd, g, tag):
            sl = slice(g * 4, g * 4 + 4)
            ve = stp.tile([P, 4], F32, name="ve", tag=f"ve{tag}")
            nc.vector.tensor_scalar(ve[:], mvg[:, sl, 1:2], 1.0, 1e-5, ALU.mult, ALU.add)
            y = rstd[:, sl]
            nc.vector.tensor_scalar(y.bitcast(I32), ve[:].bitcast(I32),
                                    1, None, ALU.logical_shift_right)
            nc.vector.tensor_scalar(y.bitcast(I32), y.bitcast(I32),
                                    1597463007, -1, ALU.subtract, ALU.mult)
            for _ in range(2):
                t1 = stp.tile([P, 4], F32, name="nt", tag=f"nt{tag}")
                nc.vector.tensor_mul(t1[:], y, y)
                nc.vector.tensor_mul(t1[:], t1[:], ve[:])
                nc.vector.tensor_scalar(t1[:], t1[:], -0.5, 1.5, ALU.mult, ALU.add)
                nc.vector.tensor_mul(y, y, t1[:])

        def ln_apply(dst_slice, src_slice, mvg, rstd, tt):
            nb = stp.tile([P, 1], F32, name="nb", tag=f"nb{tt % 2}")
            nc.vector.scalar_tensor_tensor(nb[:], mvg[:, tt, 0:1], -1.0,
                                           rstd[:, tt:tt + 1], ALU.mult, ALU.mult)
            nc.scalar.activation(dst_slice, src_slice, AF.Identity,
                                 bias=nb[:], scale=rstd[:, tt:tt + 1])

        def transpose_tile(hT, src, tpool, tt):
            for ct in range(KC):
                tps = tpool.tile([P, P], BF16, name="tp")
                nc.tensor.transpose(tps[:], src[:, tt, ct * P:(ct + 1) * P], ident_b[:])
                if (tt + ct) % 2:
                    nc.vector.tensor_copy(hT[:, ct, tt * P:(tt + 1) * P], tps[:])
                else:
                    nc.scalar.copy(hT[:, ct, tt * P:(tt + 1) * P], tps[:])

        # shift permute between A-order and B-order token layouts (a-major for
        # fine-grained downstream unlock)
        def permute(dstT, srcT, fwd, sas=None):
            G = IMG // WS  # 4
            sv = srcT[:].rearrange("p k (a b i j) -> p k a b i j", a=G, b=G, i=WS, j=WS)
            dv = dstT[:].rearrange("p k (a b i j) -> p k a b i j", a=G, b=G, i=WS, j=WS)
            for a in range(G):
                for qa in range(2):
                    for qb in range(2):
                        di = slice(0, 4) if qa == 0 else slice(4, 8)
                        si = slice(4, 8) if qa == 0 else slice(0, 4)
                        dj = slice(0, 4) if qb == 0 else slice(4, 8)
                        sj = slice(4, 8) if qb == 0 else slice(0, 4)
                        sa = (a + qa) % G
                        if sas is not None and (sa if fwd else a) not in sas:
                            continue
                        if qb == 0:
                            bpairs = [(slice(0, G), slice(0, G))]
                        else:
                            bpairs = [(slice(0, G - 1), slice(1, G)), (slice(G - 1, G), slice(0, 1))]
                        for db, sb_ in bpairs:
                            for ct in range(KC):
                                eng = (nc.gpsimd, nc.vector)[(a + ct) % 2]
                                if fwd:
                                    eng.tensor_copy(dv[:, ct, a, db, di, dj],
                                                    sv[:, ct, sa, sb_, si, sj])
                                else:
                                    eng.tensor_copy(dv[:, ct, sa, sb_, si, sj],
                                                    sv[:, ct, a, db, di, dj])

        def emit_ln1_t1(hctx, tpool, half):
            if half == 0:
                hctx['h'] = hp.tile([P, NT, C], BF16, name="h")
                hctx['mvg'] = stp.tile([P, NT, 2], F32, name="mvg1", tag="mvg1")
                hctx['rstd'] = stp.tile([P, NT], F32, name="rstd1", tag="rstd1")
                hctx['hT_A'] = hTp.tile([P, KC, L], BF16, name="hT")
            h, mvg, rstd, hT_A = hctx['h'], hctx['mvg'], hctx['rstd'], hctx['hT_A']
            for tt in range(half * 4, half * 4 + 4):
                ln_stats(x[:, tt], mvg, tt)
            ln_rstd_group(mvg, rstd, half, f"l1{half}")
            for tt in range(half * 4, half * 4 + 4):
                ln_apply(h[:, tt], x[:, tt], mvg, rstd, tt)
                transpose_tile(hT_A, h, tpool, tt)

        def finalize_hT(hctx, shifted):
            if shifted:
                hT = hTp.tile([P, KC, L], BF16, name="hT")
                permute(hT, hctx['hT_A'], True)
                return hT
            return hctx['hT_A']

        # ---------- dec ----------
        with tc.tile_pool(name="decp", bufs=1) as decp, \
             tc.tile_pool(name="dps", bufs=2, space="PSUM") as dps, \
             tc.tile_pool(name="tp0", bufs=2, space="PSUM") as tp0:
            xT_sb = decp.tile([P, KE, L], BF16)
            nc.sync.dma_start(xT_sb[:], dr['xT'].rearrange("(k p) t -> p k t", p=P))
            decw_sb = decp.tile([P, KE, C], BF16)
            nc.sync.dma_start(decw_sb[:], dr['decw'].rearrange("(k p) c -> p k c", p=P))
            for tt in range(NT):
                pos_t = decp.tile([P, C], F32, name="pos_t", tag="pos", bufs=2)
                nc.sync.dma_start(pos_t[:], dr['posb'][tt * P:(tt + 1) * P, :])
                ps = dps.tile([P, C], F32)
                for kk in range(KE):
                    nc.tensor.matmul(ps[:], xT_sb[:, kk, tt * P:(tt + 1) * P],
                                     decw_sb[:, kk, :], start=(kk == 0), stop=(kk == KE - 1))
                nc.vector.tensor_add(x[:, tt], ps[:], pos_t[:])
            hctx0 = {}
            emit_ln1_t1(hctx0, tp0, 0)
            emit_ln1_t1(hctx0, tp0, 1)
            pending_hT = finalize_hT(hctx0, False)

        # ---------- layers ----------
        for i in range(D):
            shift = (i % 2) == 1
            shift_next = ((i + 1) % 2) == 1
            hT = pending_hT
            # qk + v
            qkT = qkp.tile([P, 8, L], BF16, name="qkT")
            bqk_sb = bp.tile([P, 8], F32, name="bqk", tag="bqk")
            nc.sync.dma_start(bqk_sb[:], dr['bqk'][i])
            with tc.tile_pool(name="mmps1", bufs=2, space="PSUM") as mmps, \
                 tc.tile_pool(name="vps", bufs=2, space="PSUM") as vps:
                for mo in range(8):
                    wqk_c = wqkp.tile([P, KC, P], BF16, name="wqkc")
                    nc.sync.dma_start(wqk_c[:], dr['wqk'][i][:, mo * P:(mo + 1) * P]
                                      .rearrange("(k p) m -> p k m", p=P))
                    for tc2 in range(2):
                        ps = mmps.tile([P, C], F32, name="mm")
                        for kk in range(KC):
                            nc.tensor.matmul(ps[:], wqk_c[:, kk, :],
                                             hT[:, kk, tc2 * 512:(tc2 + 1) * 512],
                                             start=(kk == 0), stop=(kk == KC - 1))
                        nc.scalar.activation(qkT[:, mo, tc2 * 512:(tc2 + 1) * 512], ps[:],
                                             AF.Identity, bias=bqk_sb[:, mo:mo + 1])
                v_aug = vp.tile([P, NT, VW], BF16, name="vaug")
                wvp_sb = wvpp.tile([P, KC, VW], BF16, name="wvp")
                nc.sync.dma_start(wvp_sb[:], dr['wvp'][i].rearrange("(k p) m -> p k m", p=P))
                vb_sb = bp.tile([P, VW], BF16, name="vb", tag="vb", bufs=1)
                nc.sync.dma_start(vb_sb[:], dr['vb'][i])
                if not shift:
                    for tt in range(NT):
                        ps = vps.tile([P, VW], F32, name="vps")
                        for kk in range(KC):
                            nc.tensor.matmul(ps[:, 0:512], hT[:, kk, tt * P:(tt + 1) * P],
                                             wvp_sb[:, kk, 0:512], start=(kk == 0),
                                             stop=(kk == KC - 1), skip_group_check=True)
                            nc.tensor.matmul(ps[:, 512:VW], hT[:, kk, tt * P:(tt + 1) * P],
                                             wvp_sb[:, kk, 512:VW], start=(kk == 0),
                                             stop=(kk == KC - 1), skip_group_check=True)
                        nc.vector.tensor_add(v_aug[:, tt], ps[:], vb_sb[:])

            # attention + T2 + proj/LN2 + MLP (fc1 interleaved into attention window)
            att = attp.tile([P, NT, C], BF16, name="att")
            mvg2 = stp.tile([P, NT, 2], F32, name="mvg2", tag="mvg2")
            rstd2 = stp.tile([P, NT], F32, name="rstd2", tag="rstd2")
            h2 = hp.tile([P, NT, C], BF16, name="h")
            h2T = hTp.tile([P, KC, L], BF16, name="hT")
            aT_B = hTp.tile([P, KC, L], BF16, name="hT")
            wp_sb = wpp.tile([P, KC, C], BF16, name="wp")
            nc.sync.dma_start(wp_sb[:], dr['wp'][i].rearrange("(k p) m -> p k m", p=P))
            if not skip_bias:
                pb_sb = bp.tile([P, C], F32, name="pb", tag="pb")
                nc.sync.dma_start(pb_sb[:], dr['pb'][i])
                f2b_sb = bp.tile([P, C], F32, name="f2b", tag="f2b")
                nc.sync.dma_start(f2b_sb[:], dr['f2b'][i])
            f1b_sb = bp.tile([P, FH // P], F32, name="f1b", tag="f1b")
            nc.sync.dma_start(f1b_sb[:], dr['f1b'][i])
            f1cs, f2cs, gs = {}, {}, {}
            hctx = {}

            with tc.tile_pool(name="sps", bufs=1, space="PSUM") as sps, \
                 tc.tile_pool(name="avps", bufs=1, space="PSUM") as avps, \
                 tc.tile_pool(name="tpx", bufs=2, space="PSUM") as tpx, \
                 ExitStack() as LS:
                lp = {}

                def attn_wp2(wp2, av_pools=None):
                    pts = []
                    for rg in range(4):
                        abt = abp.tile([P, 4 * NW], BF16, name=f"ab{rg % 2}")
                        nc.sync.dma_start(abt[:], dr['ab'][i, wp2, rg])
                        sp = sps.tile([P, 4, NW], F32, name=f"s{rg % 2}", tag=f"s{rg % 2}")
                        spf = sp[:].rearrange("p a b -> p (a b)")
                        nc.tensor.matmul(spf, ident_b[:], abt[:],
                                         start=True, stop=False, skip_group_check=True)
                        for hi in range(4):
                            for w01 in range(2):
                                qs = qkT[rg * HD:(rg + 1) * HD, hi,
                                         (wp2 * 2 + w01) * NW:(wp2 * 2 + w01 + 1) * NW]
                                ks = qkT[rg * HD:(rg + 1) * HD, 4 + hi,
                                         (wp2 * 2 + w01) * NW:(wp2 * 2 + w01 + 1) * NW]
                                nc.tensor.matmul(sp[w01 * NW:(w01 + 1) * NW, hi, :], ks, qs,
                                                 start=False,
                                                 stop=(hi == 3 and w01 == 1),
                                                 tile_position=(rg * HD, w01 * NW),
                                                 skip_group_check=True)
                        pt = ppool.tile([P, 4, NW], BF16, name=f"p{rg}", tag=f"p{rg}")
                        nc.scalar.activation(pt[:].rearrange("p a b -> p (a b)"), spf, AF.Exp)
                        pts.append(pt)
                    for half in range(2):
                        avp_h = av_pools[half] if av_pools else avps
                        av = avp_h.tile([P, 8, VBLK], F32, name="av", tag="av")
                        for w01 in range(2):
                            rows = slice(w01 * NW, (w01 + 1) * NW)
                            for hh in range(8):
                                hglob = half * 8 + hh
                                hi, rg = hglob // 4, hglob % 4
                                nc.tensor.matmul(
                                    av[rows, hh, :], pts[rg][rows, hi, :],
                                    v_aug[rows, wp2, hglob * VBLK:(hglob + 1) * VBLK],
                                    start=True, stop=True,
                                    tile_position=(w01 * NW, w01 * NW))
                        rec = recp.tile([P, 8], F32, name=f"rec{half}", tag=f"rec{half}")
                        nc.vector.reciprocal(rec[:], av[:, :, HD])
                        rb = rec[:].rearrange("p (a b) -> p a b", b=1).to_broadcast((P, 8, HD))
                        dst = att[:, wp2, half * 256:(half + 1) * 256] \
                            .rearrange("p (a b) -> p a b", b=HD)
                        nc.vector.tensor_mul(dst, av[:, :, 0:HD], rb)
                    transpose_tile(aT_B, att, tpx, wp2)

                def ln2_half(g):
                    ln_rstd_group(mvg2, rstd2, g, f"l2{g}")
                    for tt in range(g * 4, g * 4 + 4):
                        ln_apply(h2[:, tt], x[:, tt], mvg2, rstd2, tt)
                        transpose_tile(h2T, h2, tpx, tt)

                def fc1_chunk(tc2, ho_lo, ho_hi):
                    for ho in range(ho_lo, ho_hi):
                        if ho not in f1cs:
                            f1cs[ho] = f1p.tile([P, KC, P], BF16, name=f"f1c{ho}", tag=f"f1c{ho}")
                            nc.sync.dma_start(f1cs[ho][:], dr['f1'][i][:, ho * P:(ho + 1) * P]
                                              .rearrange("(k p) m -> p k m", p=P))
                            f2cs[ho] = f2p.tile([P, C], BF16, name=f"f2c{ho}", tag=f"f2c{ho}")
                            nc.sync.dma_start(f2cs[ho][:], dr['f2'][i][ho * P:(ho + 1) * P, :])
                        ps1 = lp['mmps3'].tile([P, C], F32, name="mm")
                        for kk in range(KC):
                            nc.tensor.matmul(ps1[:], f1cs[ho][:, kk, :],
                                             h2T[:, kk, tc2 * 512:(tc2 + 1) * 512],
                                             start=(kk == 0), stop=(kk == KC - 1))
                        g = gp.tile([P, C], BF16, name=f"g{ho}", tag=f"g{ho}")
                        nc.scalar.activation(g[:], ps1[:], AF.Gelu, bias=f1b_sb[:, ho:ho + 1])
                        gs[(tc2, ho)] = g

                def proj_tile(tt, aT):
                    ps = lp['mmps2'].tile([P, C], F32, name="mm")
                    for kk in range(KC):
                        nc.tensor.matmul(ps[:], aT[:, kk, tt * P:(tt + 1) * P],
                                         wp_sb[:, kk, :], start=(kk == 0), stop=(kk == KC - 1))
                    nc.vector.tensor_add(x[:, tt], ps[:], x[:, tt])
                    if not skip_bias:
                        nc.vector.tensor_add(x[:, tt], x[:, tt], pb_sb[:])
                    ln_stats(x[:, tt], mvg2, tt)

                def v_tile_ps(pool, t0):
                    ps = pool.tile([P, VW], F32, name="vps")
                    for kk in range(KC):
                        nc.tensor.matmul(ps[:, 0:512], hT[:, kk, t0 * P:(t0 + 1) * P],
                                         wvp_sb[:, kk, 0:512], start=(kk == 0),
                                         stop=(kk == KC - 1), skip_group_check=True)
                        nc.tensor.matmul(ps[:, 512:VW], hT[:, kk, t0 * P:(t0 + 1) * P],
                                         wvp_sb[:, kk, 512:VW], start=(kk == 0),
                                         stop=(kk == KC - 1), skip_group_check=True)
                    nc.vector.tensor_add(v_aug[:, t0], ps[:], vb_sb[:])

                if shift:
                    aT = hTp.tile([P, KC, L], BF16, name="hT")
                    with tc.tile_pool(name="vps2", bufs=1, space="PSUM") as vps2, \
                         tc.tile_pool(name="av2", bufs=1, space="PSUM") as av2:
                        for wp2 in range(NT):
                            if wp2 == 0:
                                v_tile_ps(vps2, 0)
                                v_tile_ps(vps2, 1)
                            attn_wp2(wp2, (avps, av2))
                            if wp2 + 2 < NT:
                                v_tile_ps(vps2, wp2 + 2)
                            if wp2 % 2 == 1:
                                permute(aT, aT_B, False, sas=(wp2 // 2,))

                lp['mmps3'] = LS.enter_context(
                    tc.tile_pool(name="mmps3", bufs=2, space="PSUM"))
                with tc.tile_pool(name="mmps2", bufs=1, space="PSUM") as mmps2:
                    lp['mmps2'] = mmps2
                    if not shift:
                        for wp2 in range(NT):
                            attn_wp2(wp2)
                            proj_tile(wp2, aT_B)
                            if wp2 == 3:
                                ln2_half(0)
                            if wp2 >= 4:
                                fc1_chunk(0, (wp2 - 4) * 4, (wp2 - 4) * 4 + 4)
                            if wp2 == 7:
                                ln2_half(1)
                    else:
                        for tt in range(NT):
                            proj_tile(tt, aT)
                            if tt == 3:
                                ln2_half(0)
                            if tt >= 4:
                                fc1_chunk(0, (tt - 4) * 4, (tt - 4) * 4 + 4)
                            if tt == 7:
                                ln2_half(1)

                with tc.tile_pool(name="fc2ps", bufs=1, space="PSUM") as fc2ps:
                    def fc2_group(tc2, j):
                        pso = fc2ps.tile([P, C], F32, name="fc2", tag="fc2")
                        for ho in range(FH // P):
                            nc.tensor.matmul(pso[:], gs[(tc2, ho)][:, j * P:(j + 1) * P],
                                             f2cs[ho][:],
                                             start=(ho == 0), stop=(ho == FH // P - 1))
                        tt = tc2 * 4 + j
                        nc.vector.tensor_add(x[:, tt], pso[:], x[:, tt])
                        if not skip_bias:
                            nc.vector.tensor_add(x[:, tt], x[:, tt], f2b_sb[:])

                    for j in range(4):
                        fc2_group(0, j)
                        fc1_chunk(1, j * 4, j * 4 + 4)
                    if i + 1 < D:
                        emit_ln1_t1(hctx, tpx, 0)
                        if shift_next:
                            hctx['hT_P'] = hTp.tile([P, KC, L], BF16, name="hT")
                            permute(hctx['hT_P'], hctx['hT_A'], True, sas=(0, 1))
                    for j in range(4):
                        fc2_group(1, j)
                    if i + 1 < D:
                        emit_ln1_t1(hctx, tpx, 1)
                        if shift_next:
                            permute(hctx['hT_P'], hctx['hT_A'], True, sas=(2, 3))
                            pending_hT = hctx['hT_P']
                        else:
                            pending_hT = hctx['hT_A']

        # ---------- final LN + gelu + pred ----------
        hf = hp.tile([P, NT, C], BF16, name="h")
        mvgf = stp.tile([P, NT, 2], F32, name="mvgf", tag="mvg1")
        rstdf = stp.tile([P, NT], F32, name="rstdf", tag="rstd1")
        for tt in range(NT):
            ln_stats(x[:, tt], mvgf, tt)
        for g in range(2):
            ln_rstd_group(mvgf, rstdf, g, f"lf{g}")
        for tt in range(NT):
            ln_apply(hf[:, tt], x[:, tt], mvgf, rstdf, tt)
        nfw_sb = bp.tile([P, KC], F32, name="nfw", tag="nfw")
        nc.sync.dma_start(nfw_sb[:], dr['nfw'])
        nfb_sb = bp.tile([P, KC], F32, name="nfb", tag="nfb")
        nc.sync.dma_start(nfb_sb[:], dr['nfb'])
        pwb_sb = bp.tile([P, N_E // P], F32, name="pwb", tag="pwb", bufs=1)
        nc.sync.dma_start(pwb_sb[:], dr['pwb'])
        with tc.tile_pool(name="tpf", bufs=2, space="PSUM") as tpool, \
             tc.tile_pool(name="mmpsf", bufs=4, space="PSUM") as mmps:
            gT = hTp.tile([P, KC, L], BF16, name="hT")
            for tt in range(NT):
                for ct in range(KC):
                    tps = tpool.tile([P, P], BF16, name="tp")
                    nc.tensor.transpose(tps[:], hf[:, tt, ct * P:(ct + 1) * P], ident_b[:])
                    nc.scalar.activation(gT[:, ct, tt * P:(tt + 1) * P], tps[:], AF.Gelu,
                                         bias=nfb_sb[:, ct:ct + 1], scale=nfw_sb[:, ct:ct + 1])
            for no in range(N_E // P):
                pwc = pwp.tile([P, KC, P], BF16, name="pwc")
                nc.sync.dma_start(pwc[:], dr['pw'][:, no * P:(no + 1) * P]
                                  .rearrange("(k p) m -> p k m", p=P))
                for tc2 in range(2):
                    ps = mmps.tile([P, 512], F32, name="mm")
                    for kk in range(KC):
                        nc.tensor.matmul(ps[:], pwc[:, kk, :],
                                         gT[:, kk, tc2 * 512:(tc2 + 1) * 512],
                                         start=(kk == 0), stop=(kk == KC - 1))
                    osb = outp.tile([P, 512], F32, name="osb")
                    nc.scalar.activation(osb[:], ps[:], AF.Identity, bias=pwb_sb[:, no:no + 1])
                    nc.sync.dma_start(outT[no * P:(no + 1) * P, tc2 * 512:(tc2 + 1) * 512], osb[:])

    nc.compile()
    _BUILD_CACHE[key] = nc
    return nc


LAST_RESULTS = None


def kernel(**inputs):
    global LAST_RESULTS
    from concourse import bass_utils
    sh, xts = _prepare(inputs)
    skip_bias = bool(np.all(sh['pb'] == 0) and np.all(sh['f2b'] == 0))
    nc = _build(skip_bias)
    in_maps = []
    for c in range(_NCORES):
        m = dict(sh)
        m['xT'] = xts[c % B]
        in_maps.append(m)
    trace = os.environ.get("BT_TRACE", "0") == "1"
    if trace:
        try:
            import antenv.axon_hooks  # noqa: F401
        except ImportError:
            trace = False
    res = bass_utils.run_bass_kernel_spmd(nc, in_maps, core_ids=list(range(_NCORES)),
                                          trace=trace)
    LAST_RESULTS = res
    outs = []
    for c in range(B):
        oT = res.results[c % _NCORES]['outT']  # [N_E, L] in device token order
        o = oT.T[_WM_INV]                      # [L, N_E] raster order
        outs.append(o)
    return np.stack(outs).astype(np.float32)
